# revision 11
# baseline (speedup 1.0000x reference)
"""Trainium2 Bass kernel for nn_BinRegularizer (histogram_binning).

Strategy (v2, host-scheduled quantized sums):
The reference's f32 sequential segment_sum quantizes each element's
contribution to the current accumulator ulp u.  Instead of computing
per-chunk tables on a static ladder of dyadic grids (baseline: ~53
full-tile ops), the host predicts each bin-accumulator's binade
trajectory from a 1/1024 subsample (self-consistent iteration over
per-binade gain estimates), and the device computes per-chunk masked
quantized sums at just TWO grids per (bin, edge): the predicted binade
u_hat(c) and a directional neighbor (covers prediction error of +-1
binade near crossings).  Per-chunk constants ride in [P,1] scalar
operands (chunks == partition rows).  The host replay then picks
between the two columns data-dependently, reproducing the reference
trajectory chunk-by-chunk.

Diagnostics (mean |w-wq|, mean (w-wq)^2) come from a fused d-route:
idx = round(clip(w/a)) via magic-add, d = w - a*idx, ACT Abs/Square
accumulations -- replacing the baseline's 11 exact-stat ops.

Self-contained: hardcodes shapes (4096x16384 f32 weights, alpha[1]),
8 NeuronCores, contiguous 8M-element shards per core.
"""
import sys

sys.path.insert(0, "/opt/trn_rl_repo")

import numpy as np

f32 = np.float32

P = 128
F = 2048
NT = 32
NCORES = 8
CORE_ELEMS = P * F * NT
N_TOTAL = CORE_ELEMS * NCORES
NCHUNK = NCORES * NT * P          # 32768 chunks of 2048, stream order
CORE_CHUNKS = NT * P

LG_EXACT = -40                    # grids below any f32 ulp: qz == identity
SUBSTRIDE = 1024

# schedule names: (kind, bins, edges). s-side on w, q-side on z=w|w|.
SCHEDS = ["s01", "s3", "q03", "q1", "q2"]

# emits: name -> (sched, y-kind, edge, side). side: 'lo' (sum below edge)
# or 'hi' (sum above edge). edge is the threshold id.
EMITS = [
    ("es0", "s01", "th1", "lo"),
    ("es1", "s01", "th2", "lo"),
    ("es3", "s3", "th3", "hi"),
    ("eq0", "q03", "m1", "lo"),
    ("eq1m1", "q1", "m1", "lo"),
    ("eq1m2", "q1", "m2", "lo"),
    ("eq2", "q2", "p2", "lo"),    # fold: on |z| = w^2 grid
    ("eq3", "q03", "p2", "hi"),
]
# emits computed on DVE as stt(y, ST, zeros, sub, min) (value = sum min(y-st,0));
# the rest on ACT as Relu(bias +- y) with accum (value = sum relu(st-y) = -min-sum
# for 'lo' with scale=-1, or sum relu(y-st) for 'hi' with scale=+1, bias=-st).
DVE_EMITS = {"es0", "es1"}
POOL_EMITS = set()

NG_SLOTS = 8
# per-tile const columns: 10 M cols (5 scheds x 2 widths) + 16 bias cols
CT_COLS = 26

_CACHE = {}


def MS(u):
    return f32(f32(3.0 * 2.0**22) * f32(u))


def _qz_of(x, u):
    m = MS(u)
    return f32(f32(f32(x) + m) - m)


def _build_program(repeat=1):
    import concourse.bacc as bacc
    import concourse.tile as tile
    from concourse import mybir

    AL = mybir.AluOpType
    AF = mybir.ActivationFunctionType
    DT = mybir.dt.float32

    # const slot bookkeeping
    gslot = {"RINV": 0, "NEGA": 1, "TH1": 2, "TH2": 3, "TH3": 4}
    tcol = {}

    def tc_(name):
        if name not in tcol:
            tcol[name] = len(tcol)
        return tcol[name]

    for sc in SCHEDS:
        tc_(f"M_{sc}_a")
        tc_(f"M_{sc}_b")
    for nm, sc, edge, side in EMITS:
        tc_(f"B_{nm}_a")
        tc_(f"B_{nm}_b")
    assert len(tcol) <= CT_COLS, len(tcol)

    layout_d = {}
    layout_a = {}
    layout_p = {}
    for nm in ("C1", "C2", "C3"):
        layout_d[nm] = len(layout_d)
    for nm, sc, edge, side in EMITS:
        for wd in ("a", "b"):
            full = f"{nm}_{wd}"
            if nm in DVE_EMITS:
                layout_d[full] = len(layout_d)
            elif nm in POOL_EMITS:
                layout_p[full] = len(layout_p)
            else:
                layout_a[full] = len(layout_a)
    layout_a["SAD"] = len(layout_a)
    layout_a["SSQ"] = len(layout_a)
    NQ_D = len(layout_d)
    NQ_A = len(layout_a)
    NQ_P = len(layout_p)

    nc = bacc.Bacc("TRN2", target_bir_lowering=False, debug=False,
                   num_devices=NCORES)
    W = nc.dram_tensor("w", [NT * P, F], DT, kind="ExternalInput")
    CG = nc.dram_tensor("cg", [P, NG_SLOTS], DT, kind="ExternalInput")
    CT = nc.dram_tensor("ct", [P, CT_COLS * NT], DT, kind="ExternalInput")
    OUTD = nc.dram_tensor("outd", [P, NQ_D * NT], DT, kind="ExternalOutput")
    OUTA = nc.dram_tensor("outa", [P, NQ_A * NT], DT, kind="ExternalOutput")
    OUTP = (nc.dram_tensor("outp", [P, NQ_P * NT], DT,
                            kind="ExternalOutput") if NQ_P else None)
    Wv = W[:, :].rearrange("(t p) f -> t p f", p=P)

    with tile.TileContext(nc) as tc:
        with tc.tile_pool(name="wp", bufs=3) as wpool, \
             tc.tile_pool(name="yp", bufs=4) as ypool, \
             tc.tile_pool(name="zp", bufs=2) as zpool, \
             tc.tile_pool(name="dp", bufs=4) as dpool, \
             tc.tile_pool(name="singles", bufs=1) as singles:
            cgd = singles.tile([P, NG_SLOTS], DT)
            cga = singles.tile([P, NG_SLOTS], DT)
            ctd = singles.tile([P, CT_COLS * NT], DT)
            cta = singles.tile([P, CT_COLS * NT], DT)
            zeros = singles.tile([P, F], DT)
            std = singles.tile([P, NQ_D * NT], DT)
            sta = singles.tile([P, NQ_A * NT], DT)
            stp = singles.tile([P, NQ_P * NT], DT) if NQ_P else None
            gd = singles.tile([P, F], DT)
            ga = singles.tile([P, F], DT)
            gp = singles.tile([P, F], DT) if NQ_P else None

            nc.sync.dma_start(out=cgd, in_=CG[:, :])
            nc.sync.dma_start(out=ctd, in_=CT[:, :])
            nc.scalar.copy(out=cga, in_=cgd)
            nc.scalar.copy(out=cta, in_=ctd)
            nc.vector.memset(zeros, 0.0)

            def g_d(nm):
                return cgd[:, gslot[nm]:gslot[nm] + 1]

            def ct_d(nm, t):
                j = tcol[nm]
                return ctd[:, j * NT + t:j * NT + t + 1]

            def ct_a(nm, t):
                j = tcol[nm]
                return cta[:, j * NT + t:j * NT + t + 1]

            def st(name, t):
                if name in layout_d:
                    q = layout_d[name]
                    return std[:, q * NT + t:q * NT + t + 1]
                if name in layout_p:
                    q = layout_p[name]
                    return stp[:, q * NT + t:q * NT + t + 1]
                q = layout_a[name]
                return sta[:, q * NT + t:q * NT + t + 1]

            for t_rep in range(NT * repeat):
                t = t_rep % NT
                w = wpool.tile([P, F], DT, tag="w")
                nc.sync.dma_start(out=w, in_=Wv[t])

                # counts
                for nm, cn, op in (("C1", "TH1", AL.is_gt),
                                   ("C2", "TH2", AL.is_ge),
                                   ("C3", "TH3", AL.is_gt)):
                    nc.vector.tensor_scalar(
                        out=gd[:, :], in0=w[:, :], scalar1=g_d(cn),
                        scalar2=None, op0=op, op1=AL.add,
                        accum_out=st(nm, t))

                # d-route: idx = round(clip(w/a, -2, 1)); d = w - a*idx
                tt = dpool.tile([P, F], DT, tag="d")
                nc.vector.tensor_scalar(
                    out=tt[:, :], in0=w[:, :], scalar1=g_d("RINV"),
                    scalar2=-2.0, op0=AL.mult, op1=AL.max)
                im = dpool.tile([P, F], DT, tag="d")
                nc.vector.tensor_scalar(
                    out=im[:, :], in0=tt[:, :], scalar1=1.0,
                    scalar2=12582912.0, op0=AL.min, op1=AL.add)
                idx = dpool.tile([P, F], DT, tag="d")
                nc.vector.tensor_scalar(
                    out=idx[:, :], in0=im[:, :], scalar1=12582912.0,
                    scalar2=None, op0=AL.subtract)
                dt_ = dpool.tile([P, F], DT, tag="d")
                nc.vector.scalar_tensor_tensor(
                    out=dt_[:, :], in0=idx[:, :], scalar=g_d("NEGA"),
                    in1=w[:, :], op0=AL.mult, op1=AL.add)
                nc.scalar.activation(out=ga[:, :], in_=dt_[:, :], func=AF.Abs,
                                     bias=0.0, scale=1.0,
                                     accum_out=st("SAD", t))
                nc.scalar.activation(out=ga[:, :], in_=dt_[:, :],
                                     func=AF.Square, bias=0.0, scale=1.0,
                                     accum_out=st("SSQ", t))

                # z = w * |w|; azt = |z| = w*w (exact: same rne magnitude)
                absw = zpool.tile([P, F], DT, tag="absw")
                nc.vector.scalar_tensor_tensor(
                    out=absw[:, :], in0=w[:, :], scalar=-1.0,
                    in1=w[:, :], op0=AL.mult, op1=AL.max)
                z = zpool.tile([P, F], DT, tag="z")
                nc.vector.tensor_mul(out=z[:, :], in0=w[:, :], in1=absw[:, :])
                azt = zpool.tile([P, F], DT, tag="azt")
                nc.scalar.activation(out=azt[:, :], in_=w[:, :],
                                     func=AF.Square, bias=0.0, scale=1.0)

                # y tiles per (sched, width)
                ytiles = {}
                for sc in SCHEDS:
                    ysrc = w if sc.startswith("s") else (azt if sc == "q2"
                                                         else z)
                    for wd in ("a", "b"):
                        y = ypool.tile([P, F], DT, tag="y")
                        nc.vector.tensor_scalar(
                            out=y[:, :], in0=ysrc[:, :],
                            scalar1=ct_d(f"M_{sc}_{wd}", t),
                            scalar2=None, op0=AL.add)
                        ytiles[(sc, wd)] = y

                # emits
                for nm, sc, edge, side in EMITS:
                    for wd in ("a", "b"):
                        y = ytiles[(sc, wd)]
                        full = f"{nm}_{wd}"
                        if nm in DVE_EMITS or nm in POOL_EMITS:
                            eng = (nc.vector if nm in DVE_EMITS
                                   else nc.gpsimd)
                            gout = gd if nm in DVE_EMITS else gp
                            # acc = sum min/max(y - st, 0)  (st stored as +ST)
                            eng.scalar_tensor_tensor(
                                out=gout[:, :], in0=y[:, :],
                                scalar=ct_d(f"B_{nm}_{wd}", t),
                                in1=zeros[:, :], op0=AL.subtract,
                                op1=(AL.min if side == "lo" else AL.max),
                                accum_out=st(full, t))
                        elif side == "lo":
                            # acc = sum relu(st - y)  (bias col stores +ST)
                            nc.scalar.activation(
                                out=ga[:, :], in_=y[:, :], func=AF.Relu,
                                bias=ct_a(f"B_{nm}_{wd}", t), scale=-1.0,
                                accum_out=st(full, t))
                        else:
                            # acc = sum relu(y - st)  (bias col stores -ST)
                            nc.scalar.activation(
                                out=ga[:, :], in_=y[:, :], func=AF.Relu,
                                bias=ct_a(f"B_{nm}_{wd}", t), scale=1.0,
                                accum_out=st(full, t))

            nc.sync.dma_start(out=OUTD[:, :], in_=std)
            nc.sync.dma_start(out=OUTA[:, :], in_=sta)
            if NQ_P:
                nc.sync.dma_start(out=OUTP[:, :], in_=stp)

    nc.compile()
    return nc, gslot, tcol, layout_d, layout_a, layout_p


def _get_program(repeat=1):
    key = f"prog{repeat}"
    if key not in _CACHE:
        _CACHE[key] = _build_program(repeat)
    return _CACHE[key]


# ---------------- host scheduling ----------------

def _thresholds(a):
    return (f32(f32(-1.5) * a), f32(f32(-0.5) * a), f32(f32(0.5) * a))


def _predict_lgs(w_full, a):
    """Per-chunk predicted accumulator binade (log2 ulp) per (kind, bin).
    Self-consistent iteration over subsample-estimated per-binade gains."""
    th1, th2, th3 = _thresholds(a)
    sub = w_full[::SUBSTRIDE]
    c1 = sub > th1
    c2 = sub >= th2
    c3 = sub > th3
    kidx = c1.astype(np.int8) + c2.astype(np.int8) + c3.astype(np.int8)
    zsub = (sub * np.abs(sub)).astype(f32)

    preds = {}
    for kind in ("s", "q"):
        x_all = sub if kind == "s" else np.abs(zsub)
        for k in ((0, 1, 3) if kind == "s" else (0, 1, 2, 3)):
            xk = x_all[kidx == k]
            frac = float((kidx == k).mean())
            g_exact = float(xk.astype(np.float64).mean()) * frac * F
            gains = {}
            for e in range(-30, -1):
                m = MS(2.0 ** e)
                q = ((xk.astype(f32) + m) - m).astype(np.float64)
                gains[e] = float(q.mean()) * frac * F
            Pacc = 0.0
            lgs = np.empty(NCHUNK, np.int64)
            for c in range(NCHUNK):
                ap = abs(Pacc)
                lg = LG_EXACT if ap == 0.0 else max(
                    int(np.floor(np.log2(ap))) - 23, LG_EXACT)
                lgs[c] = lg
                if lg < -30:
                    Pacc += g_exact
                else:
                    Pacc += gains[min(lg, -2)]
            preds[(kind, k)] = lgs
    return preds


def _directional(lgs):
    """Second-column binade per chunk: +1 within 8% of segment length before
    each predicted upward crossing, else -1."""
    lg2 = lgs - 1
    # find crossing indices (where lgs increases)
    cross = np.nonzero(np.diff(lgs) > 0)[0] + 1  # first index of new binade
    starts = np.concatenate([[0], cross])
    ends = np.concatenate([cross, [NCHUNK]])
    for s, e in zip(starts, ends):
        if e < NCHUNK:
            wwin = max(int(0.08 * (e - s)), 16)
            lo = max(s, e - wwin)
            lg2[lo:e] = lgs[lo:e] + 1
    return lg2


def _sched_grids(w_full, a):
    preds = _predict_lgs(w_full, a)
    grids = {}
    for key, lgs in preds.items():
        grids[key] = (lgs, _directional(lgs))
    return grids


def kernel(weights, alpha):
    from concourse.bass_utils import run_bass_kernel_spmd

    w_full = np.ascontiguousarray(weights, dtype=np.float32).reshape(-1)
    a = f32(np.asarray(alpha, dtype=np.float32).reshape(-1)[0])
    assert w_full.size == N_TOTAL

    nc, gslot, tcol, layout_d, layout_a, layout_p = _get_program()
    th1, th2, th3 = _thresholds(a)
    tau1 = f32(th1 * th1)
    tau2 = f32(th3 * th3)

    grids = _sched_grids(w_full, a)
    # map sched name -> (kind, bin)
    sched_key = {"s01": ("s", 0), "s3": ("s", 3),
                 "q03": ("q", 3), "q1": ("q", 1), "q2": ("q", 2)}
    edge_th = {"th1": th1, "th2": th2, "th3": th3,
               "m1": f32(-tau1), "m2": f32(-tau2), "p2": tau2}

    # per-chunk u arrays and const columns
    ucols = {}    # (sched, wd) -> u per chunk (float64)
    ccols = {}    # colname -> per-chunk f32 value
    for sc in SCHEDS:
        lgs_a, lgs_b = grids[sched_key[sc]]
        for wd, lgs in (("a", lgs_a), ("b", lgs_b)):
            u = np.exp2(lgs.astype(np.float64))
            ucols[(sc, wd)] = u
            ccols[f"M_{sc}_{wd}"] = MS(np.exp2(lgs.astype(f32)))
    for nm, sc, edge, side in EMITS:
        t = edge_th[edge]
        for wd in ("a", "b"):
            u = ucols[(sc, wd)].astype(f32)
            m = (f32(3.0 * 2.0**22) * u).astype(f32)
            qt = ((f32(t) + m) - m).astype(f32)
            stv = (m + qt).astype(f32)
            on_act = nm not in DVE_EMITS and nm not in POOL_EMITS
            ccols[f"B_{nm}_{wd}"] = (-stv if (side == "hi" and on_act)
                                     else stv)

    gvals = np.zeros(NG_SLOTS, f32)
    gvals[gslot["RINV"]] = f32(1.0) / a
    gvals[gslot["NEGA"]] = -a
    gvals[gslot["TH1"]] = th1
    gvals[gslot["TH2"]] = th2
    gvals[gslot["TH3"]] = th3
    cg_np = np.ascontiguousarray(np.broadcast_to(gvals, (P, NG_SLOTS)))

    in_maps = []
    for c in range(NCORES):
        shard = w_full[c * CORE_ELEMS:(c + 1) * CORE_ELEMS].reshape(NT * P, F)
        ct = np.zeros((P, CT_COLS * NT), f32)
        sl = slice(c * CORE_CHUNKS, (c + 1) * CORE_CHUNKS)
        for nm, j in tcol.items():
            ct[:, j * NT:(j + 1) * NT] = ccols[nm][sl].reshape(NT, P).T
        in_maps.append({"w": shard, "cg": cg_np,
                        "ct": np.ascontiguousarray(ct)})

    res = run_bass_kernel_spmd(nc, in_maps, core_ids=list(range(NCORES)))

    dev = {}
    for lay, key in ((layout_d, "outd"), (layout_a, "outa"),
                     (layout_p, "outp")):
        nq = len(lay)
        if nq == 0:
            continue
        for qname, qi in lay.items():
            arr = np.empty(NCHUNK, np.float64)
            for c in range(NCORES):
                block = res.results[c][key].reshape(P, nq, NT)
                arr[c * CORE_CHUNKS:(c + 1) * CORE_CHUNKS] = \
                    block[:, qi, :].T.reshape(-1)
            dev[qname] = arr

    return _finish(dev, ucols, a)


def _acc_lo(dev, nm, wd):
    """sum min(y-st,0) per chunk from device accums."""
    v = dev[f"{nm}_{wd}"]
    return v if (nm in DVE_EMITS or nm in POOL_EMITS) else -v


def _finish(dev, ucols, a):
    th1, th2, th3 = _thresholds(a)
    tau1 = f32(th1 * th1)
    tau2 = f32(th3 * th3)
    lv = [f32(f32(-2) * a), f32(f32(-1) * a), f32(0.0), f32(f32(1) * a)]
    n = float(F)

    C1, C2, C3 = dev["C1"], dev["C2"], dev["C3"]

    def qt_col(sc, wd, t):
        u = ucols[(sc, wd)].astype(f32)
        m = (f32(3.0 * 2.0**22) * u).astype(f32)
        return (((f32(t) + m) - m)).astype(np.float64)

    # per-chunk per-width bin columns
    cols_s = {}
    cols_q = {}
    for wd in ("a", "b"):
        p_s0 = _acc_lo(dev, "es0", wd) + qt_col("s01", wd, th1) * (n - C1)
        p_s1t2 = _acc_lo(dev, "es1", wd) + qt_col("s01", wd, th2) * (n - C2)
        s3 = dev[f"es3_{wd}"] + qt_col("s3", wd, th3) * C3
        cols_s[(0, wd)] = p_s0
        cols_s[(1, wd)] = p_s1t2 - p_s0
        cols_s[(3, wd)] = s3

        q0 = -(_acc_lo(dev, "eq0", wd) + qt_col("q03", wd, f32(-tau1)) * (n - C1))
        q1m1 = _acc_lo(dev, "eq1m1", wd) + qt_col("q1", wd, f32(-tau1)) * (n - C1)
        q1m2 = _acc_lo(dev, "eq1m2", wd) + qt_col("q1", wd, f32(-tau2)) * (n - C2)
        q2 = _acc_lo(dev, "eq2", wd) + qt_col("q2", wd, tau2) * (C2 - C3)
        q3 = dev[f"eq3_{wd}"] + qt_col("q03", wd, tau2) * C3
        cols_q[(0, wd)] = q0
        cols_q[(1, wd)] = -(q1m2 - q1m1)
        cols_q[(2, wd)] = q2
        cols_q[(3, wd)] = q3

    sched_of = {("s", 0): "s01", ("s", 1): "s01", ("s", 3): "s3",
                ("q", 0): "q03", ("q", 1): "q1", ("q", 2): "q2",
                ("q", 3): "q03"}

    def replay(kind, k, cols):
        sc = sched_of[(kind, k)]
        lga = np.log2(ucols[(sc, "a")]).astype(np.int64)
        lgb = np.log2(ucols[(sc, "b")]).astype(np.int64)
        ca, cb = cols[(k, "a")], cols[(k, "b")]
        Pacc = 0.0
        for c in range(NCHUNK):
            ap = abs(Pacc)
            lg = -200 if ap == 0.0 else int(np.floor(np.log2(ap))) - 23
            la, lb = lga[c], lgb[c]
            # pick the available column closest to the true binade
            if abs(lg - la) <= abs(lg - lb):
                Pacc += ca[c]
            else:
                Pacc += cb[c]
        return Pacc

    s_rep = np.zeros(4)
    sq_rep = np.zeros(4)
    for k in (0, 1, 3):
        s_rep[k] = replay("s", k, cols_s)
    s_rep[2] = 0.0
    for k in range(4):
        sq_rep[k] = replay("q", k, cols_q)

    cnt_tot = np.stack([n * NCHUNK - C1.sum(), (C1 - C2).sum(),
                        (C2 - C3).sum(), C3.sum()])
    c_rep = np.minimum(cnt_tot, 2.0**24)
    levels = np.array(lv, np.float64)
    safe = np.maximum(c_rep, 1.0)
    mean = s_rep / safe
    var = sq_rep / safe - mean * mean
    total_mse = np.sum(np.where(c_rep > 0, (mean - levels) ** 2, 0.0))
    total_var = np.sum(np.where(c_rep >= 2, var, 0.0))
    loss = total_mse + total_var

    N = float(N_TOTAL)
    mean_distance = dev["SAD"].sum() / N
    quantization_mse = dev["SSQ"].sum() / N

    return np.array([loss, total_mse, total_var, quantization_mse,
                     mean_distance], np.float32)


# revision 16
# speedup vs baseline: 1.1860x; 1.1860x over previous
"""Trainium2 Bass kernel for nn_BinRegularizer (histogram_binning).

Strategy (v2, host-scheduled quantized sums):
The reference's f32 sequential segment_sum quantizes each element's
contribution to the current accumulator ulp u.  Instead of computing
per-chunk tables on a static ladder of dyadic grids (baseline: ~53
full-tile ops), the host predicts each bin-accumulator's binade
trajectory from a 1/1024 subsample (self-consistent iteration over
per-binade gain estimates), and the device computes per-chunk masked
quantized sums at just TWO grids per (bin, edge): the predicted binade
u_hat(c) and a directional neighbor (covers prediction error of +-1
binade near crossings).  Per-chunk constants ride in [P,1] scalar
operands (chunks == partition rows).  The host replay then picks
between the two columns data-dependently, reproducing the reference
trajectory chunk-by-chunk.

Diagnostics (mean |w-wq|, mean (w-wq)^2) come from a fused d-route:
idx = round(clip(w/a)) via magic-add, d = w - a*idx, ACT Abs/Square
accumulations -- replacing the baseline's 11 exact-stat ops.

Self-contained: hardcodes shapes (4096x16384 f32 weights, alpha[1]),
8 NeuronCores, contiguous 8M-element shards per core.
"""
import sys

sys.path.insert(0, "/opt/trn_rl_repo")

import numpy as np

f32 = np.float32

P = 128
F = 2048
NT = 32
NCORES = 8
CORE_ELEMS = P * F * NT
N_TOTAL = CORE_ELEMS * NCORES
NCHUNK = NCORES * NT * P          # 32768 chunks of 2048, stream order
CORE_CHUNKS = NT * P

LG_EXACT = -40                    # grids below any f32 ulp: qz == identity
SUBSTRIDE = 1024

# schedule names: (kind, bins, edges). s-side on w, q-side on z=w|w|.
SCHEDS = ["s01", "s3", "q03", "q1", "q2"]

# emits: name -> (sched, y-kind, edge, side). side: 'lo' (sum below edge)
# or 'hi' (sum above edge). edge is the threshold id.
EMITS = [
    ("es0", "s01", "th1", "lo"),
    ("es1", "s01", "th2", "lo"),
    ("es3", "s3", "th3", "hi"),
    ("eq0", "q03", "m1", "lo"),
    ("eq1m1", "q1", "m1", "lo"),
    ("eq1m2", "q1", "m2", "lo"),
    ("eq2", "q2", "p2", "lo"),    # fold: on |z| = w^2 grid
    ("eq3", "q03", "p2", "hi"),
]
# emits computed on DVE as stt(y, ST, zeros, sub, min) (value = sum min(y-st,0));
# the rest on ACT as Relu(bias +- y) with accum (value = sum relu(st-y) = -min-sum
# for 'lo' with scale=-1, or sum relu(y-st) for 'hi' with scale=+1, bias=-st).
DVE_EMITS = {"es0_a", "es0_b", "es1_a", "es1_b"}   # full column names on DVE
POOL_EMITS = set()

NG_SLOTS = 8
# per-tile const columns: 10 M cols (5 scheds x 2 widths) + 16 bias cols
CT_COLS = 26

_CACHE = {}


def MS(u):
    return f32(f32(3.0 * 2.0**22) * f32(u))


def _qz_of(x, u):
    m = MS(u)
    return f32(f32(f32(x) + m) - m)


def _build_program(repeat=1):
    import concourse.bacc as bacc
    import concourse.tile as tile
    from concourse import mybir

    AL = mybir.AluOpType
    AF = mybir.ActivationFunctionType
    DT = mybir.dt.float32

    # const slot bookkeeping
    gslot = {"RINV": 0, "NEGA": 1, "TH1": 2, "TH2": 3, "TH3": 4}
    tcol = {}

    def tc_(name):
        if name not in tcol:
            tcol[name] = len(tcol)
        return tcol[name]

    for sc in SCHEDS:
        tc_(f"M_{sc}_a")
        tc_(f"M_{sc}_b")
    for nm, sc, edge, side in EMITS:
        tc_(f"B_{nm}_a")
        tc_(f"B_{nm}_b")
    assert len(tcol) <= CT_COLS, len(tcol)

    layout_d = {}
    layout_a = {}
    layout_p = {}
    for nm in ("C1", "C2", "C3"):
        layout_d[nm] = len(layout_d)
    for nm, sc, edge, side in EMITS:
        for wd in ("a", "b"):
            full = f"{nm}_{wd}"
            if full in DVE_EMITS:
                layout_d[full] = len(layout_d)
            else:
                layout_a[full] = len(layout_a)
    layout_a["SAD"] = len(layout_a)
    layout_a["SSQ"] = len(layout_a)
    NQ_D = len(layout_d)
    NQ_A = len(layout_a)
    NQ_P = len(layout_p)

    nc = bacc.Bacc("TRN2", target_bir_lowering=False, debug=False,
                   num_devices=NCORES)
    W = nc.dram_tensor("w", [NT * P, F], DT, kind="ExternalInput")
    CG = nc.dram_tensor("cg", [P, NG_SLOTS], DT, kind="ExternalInput")
    CT = nc.dram_tensor("ct", [P, CT_COLS * NT], DT, kind="ExternalInput")
    OUTD = nc.dram_tensor("outd", [P, NQ_D * NT], DT, kind="ExternalOutput")
    OUTA = nc.dram_tensor("outa", [P, NQ_A * NT], DT, kind="ExternalOutput")
    OUTP = (nc.dram_tensor("outp", [P, NQ_P * NT], DT,
                            kind="ExternalOutput") if NQ_P else None)
    Wv = W[:, :].rearrange("(t p) f -> t p f", p=P)

    with tile.TileContext(nc) as tc:
        with tc.tile_pool(name="wp", bufs=3) as wpool, \
             tc.tile_pool(name="yp", bufs=4) as ypool, \
             tc.tile_pool(name="zp", bufs=2) as zpool, \
             tc.tile_pool(name="dp", bufs=4) as dpool, \
             tc.tile_pool(name="singles", bufs=1) as singles:
            cgd = singles.tile([P, NG_SLOTS], DT)
            cga = singles.tile([P, NG_SLOTS], DT)
            ctd = singles.tile([P, CT_COLS * NT], DT)
            cta = singles.tile([P, CT_COLS * NT], DT)
            zeros = singles.tile([P, F], DT)
            std = singles.tile([P, NQ_D * NT], DT)
            sta = singles.tile([P, NQ_A * NT], DT)
            stp = singles.tile([P, NQ_P * NT], DT) if NQ_P else None
            gd = singles.tile([P, F], DT)
            ga = singles.tile([P, F], DT)
            gp = singles.tile([P, F], DT) if NQ_P else None

            nc.sync.dma_start(out=cgd, in_=CG[:, :])
            nc.sync.dma_start(out=ctd, in_=CT[:, :])
            nc.scalar.copy(out=cga, in_=cgd)
            nc.scalar.copy(out=cta, in_=ctd)
            nc.vector.memset(zeros, 0.0)

            def g_d(nm):
                return cgd[:, gslot[nm]:gslot[nm] + 1]

            def ct_d(nm, t):
                j = tcol[nm]
                return ctd[:, j * NT + t:j * NT + t + 1]

            def ct_a(nm, t):
                j = tcol[nm]
                return cta[:, j * NT + t:j * NT + t + 1]

            def st(name, t):
                if name in layout_d:
                    q = layout_d[name]
                    return std[:, q * NT + t:q * NT + t + 1]
                if name in layout_p:
                    q = layout_p[name]
                    return stp[:, q * NT + t:q * NT + t + 1]
                q = layout_a[name]
                return sta[:, q * NT + t:q * NT + t + 1]

            for t_rep in range(NT * repeat):
                t = t_rep % NT
                w = wpool.tile([P, F], DT, tag="w")
                nc.sync.dma_start(out=w, in_=Wv[t])

                # counts
                for nm, cn, op in (("C1", "TH1", AL.is_gt),
                                   ("C2", "TH2", AL.is_ge),
                                   ("C3", "TH3", AL.is_gt)):
                    nc.vector.tensor_scalar(
                        out=gd[:, :], in0=w[:, :], scalar1=g_d(cn),
                        scalar2=None, op0=op, op1=AL.add,
                        accum_out=st(nm, t))

                # d-route: idx = round(clip(w/a, -2, 1)); d = w - a*idx
                tt = dpool.tile([P, F], DT, tag="d")
                nc.vector.tensor_scalar(
                    out=tt[:, :], in0=w[:, :], scalar1=g_d("RINV"),
                    scalar2=-2.0, op0=AL.mult, op1=AL.max)
                im = dpool.tile([P, F], DT, tag="d")
                nc.vector.tensor_scalar(
                    out=im[:, :], in0=tt[:, :], scalar1=1.0,
                    scalar2=12582912.0, op0=AL.min, op1=AL.add)
                idx = dpool.tile([P, F], DT, tag="d")
                nc.vector.tensor_scalar(
                    out=idx[:, :], in0=im[:, :], scalar1=12582912.0,
                    scalar2=None, op0=AL.subtract)
                dt_ = dpool.tile([P, F], DT, tag="d")
                nc.vector.scalar_tensor_tensor(
                    out=dt_[:, :], in0=idx[:, :], scalar=g_d("NEGA"),
                    in1=w[:, :], op0=AL.mult, op1=AL.add)
                nc.scalar.activation(out=ga[:, :], in_=dt_[:, :], func=AF.Abs,
                                     bias=0.0, scale=1.0,
                                     accum_out=st("SAD", t))
                nc.scalar.activation(out=ga[:, :], in_=dt_[:, :],
                                     func=AF.Square, bias=0.0, scale=1.0,
                                     accum_out=st("SSQ", t))

                # z = w * |w|; azt = |z| = w*w (exact: same rne magnitude)
                absw = zpool.tile([P, F], DT, tag="absw")
                nc.vector.scalar_tensor_tensor(
                    out=absw[:, :], in0=w[:, :], scalar=-1.0,
                    in1=w[:, :], op0=AL.mult, op1=AL.max)
                z = zpool.tile([P, F], DT, tag="z")
                nc.vector.tensor_mul(out=z[:, :], in0=w[:, :], in1=absw[:, :])
                azt = zpool.tile([P, F], DT, tag="azt")
                nc.scalar.activation(out=azt[:, :], in_=w[:, :],
                                     func=AF.Square, bias=0.0, scale=1.0)

                # y tiles per (sched, width)
                ytiles = {}
                for sc in SCHEDS:
                    ysrc = w if sc.startswith("s") else (azt if sc == "q2"
                                                         else z)
                    for wd in ("a", "b"):
                        y = ypool.tile([P, F], DT, tag="y")
                        nc.vector.tensor_scalar(
                            out=y[:, :], in0=ysrc[:, :],
                            scalar1=ct_d(f"M_{sc}_{wd}", t),
                            scalar2=None, op0=AL.add)
                        ytiles[(sc, wd)] = y

                # emits
                for nm, sc, edge, side in EMITS:
                    for wd in ("a", "b"):
                        y = ytiles[(sc, wd)]
                        full = f"{nm}_{wd}"
                        if full in DVE_EMITS:
                            eng = nc.vector
                            gout = gd
                            # acc = sum min/max(y - st, 0)  (st stored as +ST)
                            eng.scalar_tensor_tensor(
                                out=gout[:, :], in0=y[:, :],
                                scalar=ct_d(f"B_{nm}_{wd}", t),
                                in1=zeros[:, :], op0=AL.subtract,
                                op1=(AL.min if side == "lo" else AL.max),
                                accum_out=st(full, t))
                        elif side == "lo":
                            # acc = sum relu(st - y)  (bias col stores +ST)
                            nc.scalar.activation(
                                out=ga[:, :], in_=y[:, :], func=AF.Relu,
                                bias=ct_a(f"B_{nm}_{wd}", t), scale=-1.0,
                                accum_out=st(full, t))
                        else:
                            # acc = sum relu(y - st)  (bias col stores -ST)
                            nc.scalar.activation(
                                out=ga[:, :], in_=y[:, :], func=AF.Relu,
                                bias=ct_a(f"B_{nm}_{wd}", t), scale=1.0,
                                accum_out=st(full, t))

            nc.sync.dma_start(out=OUTD[:, :], in_=std)
            nc.sync.dma_start(out=OUTA[:, :], in_=sta)
            if NQ_P:
                nc.sync.dma_start(out=OUTP[:, :], in_=stp)

    nc.compile()
    return nc, gslot, tcol, layout_d, layout_a, layout_p


def _get_program(repeat=1):
    key = f"prog{repeat}"
    if key not in _CACHE:
        _CACHE[key] = _build_program(repeat)
    return _CACHE[key]


# ---------------- host scheduling ----------------

def _thresholds(a):
    return (f32(f32(-1.5) * a), f32(f32(-0.5) * a), f32(f32(0.5) * a))


def _predict_lgs(w_full, a):
    """Per-chunk predicted accumulator binade (log2 ulp) per (kind, bin).
    Self-consistent iteration over subsample-estimated per-binade gains."""
    th1, th2, th3 = _thresholds(a)
    sub = w_full[::SUBSTRIDE]
    c1 = sub > th1
    c2 = sub >= th2
    c3 = sub > th3
    kidx = c1.astype(np.int8) + c2.astype(np.int8) + c3.astype(np.int8)
    zsub = (sub * np.abs(sub)).astype(f32)

    preds = {}
    for kind in ("s", "q"):
        x_all = sub if kind == "s" else np.abs(zsub)
        for k in ((0, 1, 3) if kind == "s" else (0, 1, 2, 3)):
            xk = x_all[kidx == k]
            frac = float((kidx == k).mean())
            g_exact = float(xk.astype(np.float64).mean()) * frac * F
            gains = {}
            for e in range(-30, -1):
                m = MS(2.0 ** e)
                q = ((xk.astype(f32) + m) - m).astype(np.float64)
                gains[e] = float(q.mean()) * frac * F
            Pacc = 0.0
            lgs = np.empty(NCHUNK, np.int64)
            for c in range(NCHUNK):
                ap = abs(Pacc)
                lg = LG_EXACT if ap == 0.0 else max(
                    int(np.floor(np.log2(ap))) - 23, LG_EXACT)
                lgs[c] = lg
                if lg < -30:
                    Pacc += g_exact
                else:
                    Pacc += gains[min(lg, -2)]
            preds[(kind, k)] = lgs
    return preds


def _directional(lgs):
    """Second-column binade per chunk: +1 within 8% of segment length before
    each predicted upward crossing, else -1."""
    lg2 = lgs - 1
    # find crossing indices (where lgs increases)
    cross = np.nonzero(np.diff(lgs) > 0)[0] + 1  # first index of new binade
    starts = np.concatenate([[0], cross])
    ends = np.concatenate([cross, [NCHUNK]])
    for s, e in zip(starts, ends):
        if e < NCHUNK:
            wwin = max(int(0.08 * (e - s)), 16)
            lo = max(s, e - wwin)
            lg2[lo:e] = lgs[lo:e] + 1
    return lg2


def _sched_grids(w_full, a):
    preds = _predict_lgs(w_full, a)
    grids = {}
    for key, lgs in preds.items():
        grids[key] = (lgs, _directional(lgs))
    return grids


def kernel(weights, alpha):
    from concourse.bass_utils import run_bass_kernel_spmd

    w_full = np.ascontiguousarray(weights, dtype=np.float32).reshape(-1)
    a = f32(np.asarray(alpha, dtype=np.float32).reshape(-1)[0])
    assert w_full.size == N_TOTAL

    nc, gslot, tcol, layout_d, layout_a, layout_p = _get_program()
    th1, th2, th3 = _thresholds(a)
    tau1 = f32(th1 * th1)
    tau2 = f32(th3 * th3)

    grids = _sched_grids(w_full, a)
    # map sched name -> (kind, bin)
    sched_key = {"s01": ("s", 0), "s3": ("s", 3),
                 "q03": ("q", 3), "q1": ("q", 1), "q2": ("q", 2)}
    edge_th = {"th1": th1, "th2": th2, "th3": th3,
               "m1": f32(-tau1), "m2": f32(-tau2), "p2": tau2}

    # per-chunk u arrays and const columns
    ucols = {}    # (sched, wd) -> u per chunk (float64)
    ccols = {}    # colname -> per-chunk f32 value
    for sc in SCHEDS:
        lgs_a, lgs_b = grids[sched_key[sc]]
        for wd, lgs in (("a", lgs_a), ("b", lgs_b)):
            u = np.exp2(lgs.astype(np.float64))
            ucols[(sc, wd)] = u
            ccols[f"M_{sc}_{wd}"] = MS(np.exp2(lgs.astype(f32)))
    for nm, sc, edge, side in EMITS:
        t = edge_th[edge]
        for wd in ("a", "b"):
            u = ucols[(sc, wd)].astype(f32)
            m = (f32(3.0 * 2.0**22) * u).astype(f32)
            qt = ((f32(t) + m) - m).astype(f32)
            stv = (m + qt).astype(f32)
            on_act = f"{nm}_{wd}" not in DVE_EMITS
            ccols[f"B_{nm}_{wd}"] = (-stv if (side == "hi" and on_act)
                                     else stv)

    gvals = np.zeros(NG_SLOTS, f32)
    gvals[gslot["RINV"]] = f32(1.0) / a
    gvals[gslot["NEGA"]] = -a
    gvals[gslot["TH1"]] = th1
    gvals[gslot["TH2"]] = th2
    gvals[gslot["TH3"]] = th3
    cg_np = np.ascontiguousarray(np.broadcast_to(gvals, (P, NG_SLOTS)))

    in_maps = []
    for c in range(NCORES):
        shard = w_full[c * CORE_ELEMS:(c + 1) * CORE_ELEMS].reshape(NT * P, F)
        ct = np.zeros((P, CT_COLS * NT), f32)
        sl = slice(c * CORE_CHUNKS, (c + 1) * CORE_CHUNKS)
        for nm, j in tcol.items():
            ct[:, j * NT:(j + 1) * NT] = ccols[nm][sl].reshape(NT, P).T
        in_maps.append({"w": shard, "cg": cg_np,
                        "ct": np.ascontiguousarray(ct)})

    res = run_bass_kernel_spmd(nc, in_maps, core_ids=list(range(NCORES)))

    dev = {}
    for lay, key in ((layout_d, "outd"), (layout_a, "outa"),
                     (layout_p, "outp")):
        nq = len(lay)
        if nq == 0:
            continue
        for qname, qi in lay.items():
            arr = np.empty(NCHUNK, np.float64)
            for c in range(NCORES):
                block = res.results[c][key].reshape(P, nq, NT)
                arr[c * CORE_CHUNKS:(c + 1) * CORE_CHUNKS] = \
                    block[:, qi, :].T.reshape(-1)
            dev[qname] = arr

    return _finish(dev, ucols, a)


def _acc_lo(dev, nm, wd):
    """sum min(y-st,0) per chunk from device accums."""
    v = dev[f"{nm}_{wd}"]
    return v if f"{nm}_{wd}" in DVE_EMITS else -v


def _finish(dev, ucols, a):
    th1, th2, th3 = _thresholds(a)
    tau1 = f32(th1 * th1)
    tau2 = f32(th3 * th3)
    lv = [f32(f32(-2) * a), f32(f32(-1) * a), f32(0.0), f32(f32(1) * a)]
    n = float(F)

    C1, C2, C3 = dev["C1"], dev["C2"], dev["C3"]

    def qt_col(sc, wd, t):
        u = ucols[(sc, wd)].astype(f32)
        m = (f32(3.0 * 2.0**22) * u).astype(f32)
        return (((f32(t) + m) - m)).astype(np.float64)

    # per-chunk per-width bin columns
    cols_s = {}
    cols_q = {}
    for wd in ("a", "b"):
        p_s0 = _acc_lo(dev, "es0", wd) + qt_col("s01", wd, th1) * (n - C1)
        p_s1t2 = _acc_lo(dev, "es1", wd) + qt_col("s01", wd, th2) * (n - C2)
        s3 = dev[f"es3_{wd}"] + qt_col("s3", wd, th3) * C3
        cols_s[(0, wd)] = p_s0
        cols_s[(1, wd)] = p_s1t2 - p_s0
        cols_s[(3, wd)] = s3

        q0 = -(_acc_lo(dev, "eq0", wd) + qt_col("q03", wd, f32(-tau1)) * (n - C1))
        q1m1 = _acc_lo(dev, "eq1m1", wd) + qt_col("q1", wd, f32(-tau1)) * (n - C1)
        q1m2 = _acc_lo(dev, "eq1m2", wd) + qt_col("q1", wd, f32(-tau2)) * (n - C2)
        q2 = _acc_lo(dev, "eq2", wd) + qt_col("q2", wd, tau2) * (C2 - C3)
        q3 = dev[f"eq3_{wd}"] + qt_col("q03", wd, tau2) * C3
        cols_q[(0, wd)] = q0
        cols_q[(1, wd)] = -(q1m2 - q1m1)
        cols_q[(2, wd)] = q2
        cols_q[(3, wd)] = q3

    sched_of = {("s", 0): "s01", ("s", 1): "s01", ("s", 3): "s3",
                ("q", 0): "q03", ("q", 1): "q1", ("q", 2): "q2",
                ("q", 3): "q03"}

    def replay(kind, k, cols):
        sc = sched_of[(kind, k)]
        lga = np.log2(ucols[(sc, "a")]).astype(np.int64)
        lgb = np.log2(ucols[(sc, "b")]).astype(np.int64)
        ca, cb = cols[(k, "a")], cols[(k, "b")]
        Pacc = 0.0
        for c in range(NCHUNK):
            ap = abs(Pacc)
            lg = -200 if ap == 0.0 else int(np.floor(np.log2(ap))) - 23
            la, lb = lga[c], lgb[c]
            # pick the available column closest to the true binade
            if abs(lg - la) <= abs(lg - lb):
                Pacc += ca[c]
            else:
                Pacc += cb[c]
        return Pacc

    s_rep = np.zeros(4)
    sq_rep = np.zeros(4)
    for k in (0, 1, 3):
        s_rep[k] = replay("s", k, cols_s)
    s_rep[2] = 0.0
    for k in range(4):
        sq_rep[k] = replay("q", k, cols_q)

    cnt_tot = np.stack([n * NCHUNK - C1.sum(), (C1 - C2).sum(),
                        (C2 - C3).sum(), C3.sum()])
    c_rep = np.minimum(cnt_tot, 2.0**24)
    levels = np.array(lv, np.float64)
    safe = np.maximum(c_rep, 1.0)
    mean = s_rep / safe
    var = sq_rep / safe - mean * mean
    total_mse = np.sum(np.where(c_rep > 0, (mean - levels) ** 2, 0.0))
    total_var = np.sum(np.where(c_rep >= 2, var, 0.0))
    loss = total_mse + total_var

    N = float(N_TOTAL)
    mean_distance = dev["SAD"].sum() / N
    quantization_mse = dev["SSQ"].sum() / N

    return np.array([loss, total_mse, total_var, quantization_mse,
                     mean_distance], np.float32)
